# revision 25
# baseline (speedup 1.0000x reference)
"""Trainium2 Bass kernel for Llama-style GQA attention (nn_LlamaAttention).

Shapes (hardcoded): hidden [1, 2048, 2048] f32, Wq [2048, 2048],
Wk/Wv [512, 2048], Wo [2048, 2048]. 32 q heads, 8 kv heads, head_dim 64,
causal + interleaved RoPE.

Sharding: tensor-parallel over heads across 8 NeuronCores. Core c owns
q heads 4c..4c+3 (one GQA group) and kv head c. Each core computes its
q/k/v projections (output-dim shard), RoPE, causal attention for its 4
heads, and a partial output projection (Wo input-dim shard). The host
sums the 8 partial [s, m] outputs.

v2: all data in bf16 (fp32 matmuls measured 2.6 cyc/row on HW vs 1 for
bf16; also halves HBM traffic). RoPE pair-swap via DVE stream_shuffle
(replaces partition-strided SBUF-SBUF DMAs). Fast-approx reciprocal for
the softmax denominators. Output projection interleaved between the two
attention head-pairs so normalization latency hides under PE work.

On-core dataflow (seq dim on the free axis):
  hT [h, s] -> qT [256, s], kT/vT [64, s]  (bf16 matmuls, N=512 chunks)
  RoPE via stream_shuffle pair-swap + DVE combine (f32 from PSUM)
  scores sT[j, i] = kT^T q, two heads packed in the PE array (K=64 rows)
  P = exp(sT) on ScalarE (no max subtraction; scores are O(1) bounded)
  causal: lower-left block skipping + one triangular 128x128 mask
  O^T accumulation with a ones-column in V to get the softmax denominator
  normalize via reciprocal_approx_fast + gpsimd partition-broadcast
  y[s, m] = O^T^T @ Wo_shard^T partials (bf16), summed on host.
"""

import numpy as np

HIDDEN = 2048
S = 2048
NH = 32
NKV = 8
HD = 64
GROUPS = 4
N_CORES = 8
DQ = 256          # q output dims per core (4 heads x 64)
CH = 512          # seq chunk width
NCH = S // CH     # 4
KT = HIDDEN // 128  # 16 contraction tiles

_cache = {}

# even/odd partition pair swap within each 32-lane quadrant
SWAP_MASK = [i ^ 1 for i in range(32)]
IDENT_MASK = list(range(32))


def _build_program(repeat=1, dumps=False):
    import concourse.bacc as bacc
    import concourse.mybir as mybir
    import concourse.tile as tile

    f32 = mybir.dt.float32
    bf16 = mybir.dt.bfloat16
    EXP = mybir.ActivationFunctionType.Exp

    nc = bacc.Bacc("TRN2", target_bir_lowering=False, debug=False,
                   num_devices=N_CORES)

    hT = nc.declare_dram_parameter("hT", [HIDDEN, S], bf16, isOutput=False)
    wqkvT = nc.declare_dram_parameter("wqkvT", [HIDDEN, DQ + 2 * HD], bf16,
                                      isOutput=False)
    woT = nc.declare_dram_parameter("woT", [DQ, HIDDEN], bf16, isOutput=False)
    tables = nc.declare_dram_parameter("tables", [128, NCH, 4 * CH], f32,
                                       isOutput=False)
    tri2 = nc.declare_dram_parameter("tri2", [128, 256], bf16, isOutput=False)
    onesall = nc.declare_dram_parameter("onesall", [128, KT * 64], bf16,
                                        isOutput=False)
    y = nc.declare_dram_parameter("y", [S, HIDDEN], bf16, isOutput=True)
    if dumps:
        dbg_q = [nc.declare_dram_parameter(f"dbg_q{m}", [128, S], bf16,
                                           isOutput=True) for m in range(2)]
        dbg_k = nc.declare_dram_parameter("dbg_k", [128, S], bf16,
                                          isOutput=True)
        dbg_v = nc.declare_dram_parameter("dbg_v", [128, 192 * KT], bf16,
                                          isOutput=True)
        dbg_ot = [nc.declare_dram_parameter(f"dbg_ot{m}", [128, S], bf16,
                                            isOutput=True) for m in range(2)]
        dbg_pt = nc.declare_dram_parameter("dbg_pt", [128, 4096], bf16,
                                           isOutput=True)
        dbg_otp = nc.declare_dram_parameter("dbg_otp", [128, 1024], f32,
                                            isOutput=True)
        dbg_nrm = nc.declare_dram_parameter("dbg_nrm", [64, 1024], f32,
                                            isOutput=True)

    with tile.TileContext(nc) as tc:
        with (
            tc.tile_pool(name="const", bufs=1) as const,
            tc.tile_pool(name="weights", bufs=1) as wpool,
            tc.tile_pool(name="ht", bufs=1) as htp,
            tc.tile_pool(name="work", bufs=1) as work,
            tc.tile_pool(name="persist", bufs=1) as persist,
            tc.tile_pool(name="vp", bufs=1) as vp,
            tc.tile_pool(name="ptp", bufs=1) as ptp,
            tc.tile_pool(name="ppa", bufs=1, space="PSUM") as ppa,
            tc.tile_pool(name="pps", bufs=1, space="PSUM") as pps,
            tc.tile_pool(name="ppo", bufs=1, space="PSUM") as ppo,
        ):
            # ---- constants / weights ----
            t_tri2 = const.tile([128, 2, 128], bf16)
            nc.sync.dma_start(out=t_tri2, in_=tri2[:, :])

            t_wqkv = wpool.tile([128, KT, DQ + 2 * HD], bf16)
            wqkv_r = wqkvT[:, :].rearrange("(t p) o -> p t o", p=128)

            # persistent activations
            t_q = [persist.tile([128, S], bf16, tag=f"q{m}", name=f"t_q{m}")
                   for m in range(2)]
            t_k = persist.tile([128, S], bf16, tag="k")
            t_ot = [persist.tile([128, S], bf16, tag=f"ot{m}", name=f"t_ot{m}")
                    for m in range(2)]
            # v tiles: [v(0:64) | ones,zeros(64:128) | v(128:192)] per j-block
            t_v = vp.tile([128, KT, 192], bf16, tag="v")
            nc.sync.dma_start(
                out=t_v[:, :, 64:128],
                in_=onesall[:, :].rearrange("p (t o) -> p t o", o=64))

            t_wo = None

            def rope(dst, ps, cos_t, sin_t, rows, ci):
                """dst[:, chunk] = ps*cos + shuffle(ps)*sin (rows slice).

                ps is the PSUM projection result (f32); shuffle swaps
                even/odd partition pairs; sign is folded into sin_t."""
                r0, r1 = rows
                c0 = ci * CH
                swp = work.tile([128, CH], f32, tag="swp", bufs=2)
                nc.vector.stream_shuffle(swp[r0:r1], ps[r0:r1], SWAP_MASK)
                tmp1 = work.tile([128, CH], f32, tag="rc1", bufs=2)
                tmp2 = work.tile([128, CH], f32, tag="rc2", bufs=2)
                nc.vector.tensor_mul(tmp1[r0:r1], ps[r0:r1], cos_t[r0:r1, :])
                nc.vector.tensor_mul(tmp2[r0:r1], swp[r0:r1], sin_t[r0:r1, :])
                nc.vector.tensor_add(dst[r0:r1, c0:c0 + CH], tmp1[r0:r1],
                                     tmp2[r0:r1])

            def emit_A(ci):
                """Projections + RoPE + k/v prep for s-chunk ci."""
                c0 = ci * CH
                t_ht = htp.tile([128, KT, CH], bf16, tag="ht", bufs=2,
                                name=f"ht_{ci}")
                ht_src = hT[:, c0:c0 + CH].rearrange("(t p) s -> p t s", p=128)
                if ci == 0:
                    # interleave weight/activation tiles so the first
                    # matmul starts ~1us in, not after a monolithic DMA
                    for kt in range(KT):
                        nc.sync.dma_start(
                            out=t_wqkv[:, kt, :], in_=wqkv_r[:, kt, :])
                        nc.sync.dma_start(
                            out=t_ht[:, kt, :], in_=ht_src[:, kt, :])
                else:
                    nc.sync.dma_start(out=t_ht, in_=ht_src)
                # rope tables for this chunk: one fused DMA
                t_tab = work.tile([128, 4 * CH], f32, tag="tab", bufs=2)
                nc.sync.dma_start(out=t_tab, in_=tables[:, ci, :])
                t_cosk = t_tab[:, 0:CH]
                t_sink = t_tab[:, CH:2 * CH]
                t_cosq = t_tab[:, 2 * CH:3 * CH]
                t_sinq = t_tab[:, 3 * CH:4 * CH]

                # kv pass first so k/v prep overlaps the q passes
                ps_kv = ppa.tile([128, CH], f32, tag="proj", bufs=2,
                                 name=f"ps_kv{ci}")
                for kt in range(KT):
                    nc.tensor.matmul(ps_kv, t_wqkv[:, kt, 256:384],
                                     t_ht[:, kt, :],
                                     start=(kt == 0), stop=(kt == KT - 1))
                # ---- k: RoPE rows 0:64 straight from PSUM, then dup ----
                rope(t_k, ps_kv, t_cosk, t_sink, (0, 64), ci)
                nc.sync.dma_start(out=t_k[64:128, c0:c0 + CH],
                                  in_=t_k[0:64, c0:c0 + CH])
                # ---- v: bf16 copy, XBAR DMA-transpose [64,128] -> [128,64] --
                v_raw = work.tile([128, CH], bf16, tag="vraw", bufs=2)
                nc.scalar.copy(v_raw[64:128], ps_kv[64:128])
                for b in range(4):
                    jb = 4 * ci + b
                    nc.sync.dma_start_transpose(
                        t_v[:, jb, 0:64],
                        v_raw[64:128, b * 128:(b + 1) * 128])
                    nc.vector.tensor_copy(t_v[:, jb, 128:192],
                                          t_v[:, jb, 0:64])

                # ---- q passes + RoPE (2 partition tiles = 4 heads) ----
                for m in range(2):
                    ps_q = ppa.tile([128, CH], f32, tag="proj", bufs=2,
                                    name=f"ps_q{m}_{ci}")
                    for kt in range(KT):
                        nc.tensor.matmul(
                            ps_q, t_wqkv[:, kt, m * 128:(m + 1) * 128],
                            t_ht[:, kt, :],
                            start=(kt == 0), stop=(kt == KT - 1))
                    rope(t_q[m], ps_q, t_cosq, t_sinq, (0, 128), ci)

            def emit_B_pair(ci, pair):
                """Attention for i-chunk ci, one packed head pair."""
                c0 = ci * CH
                if ci == 0 and pair == 0:
                    # Wo halves load during attention (DMA slack window)
                    for p_ in range(2):
                        nc.sync.dma_start(
                            out=t_wo[:, p_, :],
                            in_=woT[:, :].rearrange(
                                "(t p) o -> p t o", p=128)[:, p_, :])
                ot = ppo.tile([128, 2 * CH], f32, tag="ot", bufs=1,
                              name=f"ot{ci}_{pair}")
                njb = 4 * ci + 4
                for jb in range(njb):
                    r = jb - 4 * ci
                    off = 128 * max(r, 0)
                    w = CH - off
                    j0 = jb * 128
                    # both heads' scores in one 2-bank tile -> one ACTIVATE
                    ps_s = pps.tile([128, 2 * CH], f32, tag="s", bufs=2,
                                    name=f"ps_s{ci}_{pair}_{jb}")
                    for h in range(2):
                        nc.tensor.matmul(
                            ps_s[:, h * CH:h * CH + w],
                            t_k[64 * h:64 * (h + 1), j0:j0 + 128],
                            t_q[pair][64 * h:64 * (h + 1),
                                      c0 + off:c0 + CH],
                            start=True, stop=True,
                            tile_position=(64 * h, 0))
                    pt = ptp.tile([128, 2 * CH], bf16, tag="pt", bufs=4)
                    ps3 = ps_s[:, :].rearrange("p (a c) -> p a c", a=2)
                    pt3 = pt[:, :].rearrange("p (a c) -> p a c", a=2)
                    nc.scalar.activation(pt3[:, :, 0:w], ps3[:, :, 0:w], EXP)
                    if r >= 0:
                        nc.vector.tensor_mul(pt3[:, :, 0:128],
                                             pt3[:, :, 0:128], t_tri2)
                    if dumps and ci == 0 and pair == 0:
                        for h in range(2):
                            nc.sync.dma_start(
                                out=dbg_pt[:, h * 2048 + jb * 512:
                                           h * 2048 + jb * 512 + w],
                                in_=pt[:, h * CH:h * CH + w])
                    nc.tensor.matmul(
                        ot[0:65, off:CH],
                        t_v[:, jb, 0:65], pt[:, 0:w],
                        start=(jb == 0), stop=(jb == njb - 1))
                    nc.tensor.matmul(
                        ot[0:128, CH + off:2 * CH],
                        t_v[:, jb, 64:192], pt[:, CH:CH + w],
                        start=(jb == 0), stop=(jb == njb - 1))
                if dumps and ci == 0 and pair == 0:
                    otp_b = work.tile([128, 2 * CH], f32, tag="otdbg", bufs=1)
                    nc.scalar.copy(otp_b, ot)
                    nc.sync.dma_start(out=dbg_otp[:, :], in_=otp_b)
                # normalize: stage each head's denominator row to a
                # partition-0-based SBUF tile (partition_broadcast and the
                # custom recip op need base partition 0), then broadcast
                for h in range(2):
                    stg = work.tile([32, CH], f32, tag="stage", bufs=4)
                    if h == 0:
                        # den row lives at PSUM partition 64: quadrant move
                        nc.vector.stream_shuffle(stg, ot[64:96, 0:CH],
                                                 IDENT_MASK)
                    else:
                        nc.vector.tensor_copy(stg[0:1], ot[0:1, CH:2 * CH])
                    recip = work.tile([32, CH], f32, tag="recip", bufs=4)
                    nc.vector.reciprocal_approx_fast(recip, stg)
                    if dumps and ci == 0 and pair == 0:
                        nc.sync.dma_start(out=dbg_nrm[32 * h:32 * h + 32,
                                                      0:CH], in_=stg)
                        nc.sync.dma_start(out=dbg_nrm[32 * h:32 * h + 32,
                                                      CH:2 * CH], in_=recip)
                    bcast = work.tile([128, CH], f32, tag="bcast", bufs=2)
                    nc.gpsimd.partition_broadcast(bcast, recip[0:1, :])
                    r0_, r1_ = (0, 64) if h == 0 else (64, 128)
                    o_sl = (ot[0:64, 0:CH] if h == 0
                            else ot[64:128, CH:2 * CH])
                    nc.vector.tensor_mul(
                        t_ot[pair][r0_:r1_, c0:c0 + CH],
                        o_sl, bcast[r0_:r1_, :])

            def emit_C(ci):
                """Partial output projection for this chunk's s-columns.
                mc handled two-at-a-time so each OT weight load serves two
                matmuls (halves LDWEIGHTS traffic)."""
                for st in range(4 * ci, 4 * ci + 4):
                    for mc0 in range(0, HIDDEN // CH, 2):
                        ps_y = pps.tile([128, 2 * CH], f32, tag="s", bufs=2,
                                        name=f"ps_y{st}_{mc0}")
                        for k in range(2):
                            for j in range(2):
                                mc = mc0 + j
                                nc.tensor.matmul(
                                    ps_y[:, j * CH:(j + 1) * CH],
                                    t_ot[k][:, st * 128:(st + 1) * 128],
                                    t_wo[:, k, mc * CH:(mc + 1) * CH],
                                    start=(k == 0), stop=(k == 1))
                        t_y = work.tile([128, 2 * CH], bf16, tag="ybounce",
                                        bufs=3)
                        nc.vector.tensor_copy(t_y, ps_y)
                        nc.sync.dma_start(
                            out=y[st * 128:(st + 1) * 128,
                                  mc0 * CH:(mc0 + 2) * CH],
                            in_=t_y)

            # Software-pipelined emission, A two chunks deep: B0(ci) uses
            # ropes finished ~1.5 iterations earlier, so the DVE rope
            # latency never gates the PE. C(ci-1) (or A(2) at ci=0) sits
            # between the B pairs so pair-0's normalization resolves
            # under matmul work.
            t_wo = wpool.tile([128, 2, HIDDEN], bf16, tag="wo")
            for rep in range(repeat):
                emit_A(0)
                emit_A(1)
                for ci in range(NCH):
                    emit_B_pair(ci, 0)
                    if ci == 0:
                        emit_A(2)
                    else:
                        emit_C(ci - 1)
                    emit_B_pair(ci, 1)
                    if ci == 1:
                        emit_A(3)
                emit_C(NCH - 1)

            if dumps:
                for m in range(2):
                    nc.sync.dma_start(out=dbg_q[m][:, :], in_=t_q[m][:, :])
                    nc.sync.dma_start(out=dbg_ot[m][:, :], in_=t_ot[m][:, :])
                nc.sync.dma_start(out=dbg_k[:, :], in_=t_k[:, :])
                nc.sync.dma_start(
                    out=dbg_v[:, :].rearrange("p (t o) -> p t o", o=192),
                    in_=t_v[:, :, :])

    nc.compile()
    return nc


def _host_inputs(hidden_states, Wq, Wk, Wv, Wo):
    import ml_dtypes
    bf16 = ml_dtypes.bfloat16

    hid = np.ascontiguousarray(hidden_states.reshape(S, HIDDEN),
                               dtype=np.float32)
    hT = np.ascontiguousarray(hid.T).astype(bf16)

    scale = HD ** -0.5
    inv = 1.0 / (10000.0 ** (np.arange(0, HD, 2, dtype=np.float64) / HD))
    t = np.arange(S, dtype=np.float64)
    freqs = np.outer(t, inv)                       # [S, 32]
    cos_sd = np.repeat(np.cos(freqs), 2, axis=1)   # [S, 64]
    sin_sd = np.repeat(np.sin(freqs), 2, axis=1)
    sign = np.tile(np.array([-1.0, 1.0]), HD // 2)
    cosT = cos_sd.T                                # [64, S]
    sinT = (sin_sd * sign).T
    cosk = np.concatenate([cosT, cosT], 0).astype(np.float32)
    sink = np.concatenate([sinT, sinT], 0).astype(np.float32)
    cosq = (cosk * scale).astype(np.float32)
    sinq = (sink * scale).astype(np.float32)

    tabs = np.zeros((128, NCH, 4 * CH), np.float32)
    for ci in range(NCH):
        sl = slice(ci * CH, (ci + 1) * CH)
        tabs[:, ci, 0:CH] = cosk[:, sl]
        tabs[:, ci, CH:2 * CH] = sink[:, sl]
        tabs[:, ci, 2 * CH:3 * CH] = cosq[:, sl]
        tabs[:, ci, 3 * CH:4 * CH] = sinq[:, sl]

    tri = np.triu(np.ones((128, 128), np.float32))
    tri2 = np.concatenate([tri, tri], axis=1).astype(bf16)   # [128, 256]
    onesall = np.zeros((128, KT, 64), np.float32)
    onesall[:, :, 0] = 1.0
    onesall = onesall.reshape(128, KT * 64).astype(bf16)

    in_maps = []
    for c in range(N_CORES):
        wq_c = Wq[DQ * c:DQ * (c + 1), :]          # [256, H]
        wk_c = Wk[HD * c:HD * (c + 1), :]          # [64, H]
        wv_c = Wv[HD * c:HD * (c + 1), :]
        wqkvT = np.ascontiguousarray(
            np.concatenate([wq_c, wk_c, wv_c], axis=0).T).astype(bf16)
        woT = np.ascontiguousarray(
            Wo[:, DQ * c:DQ * (c + 1)].T).astype(bf16)
        in_maps.append({
            "hT": hT, "wqkvT": wqkvT, "woT": woT,
            "tables": tabs,
            "tri2": tri2, "onesall": onesall,
        })
    return in_maps


def kernel(hidden_states, Wq, Wk, Wv, Wo):
    import os
    from concourse.bass_utils import run_bass_kernel_spmd

    if "nc" not in _cache:
        _cache["nc"] = _build_program()
    nc = _cache["nc"]

    trace = bool(os.environ.get("BASS_HW_TRACE"))
    in_maps = _host_inputs(hidden_states, Wq, Wk, Wv, Wo)
    res = run_bass_kernel_spmd(nc, in_maps, list(range(N_CORES)),
                               trace=trace)
    if trace:
        _cache["exec_time_ns"] = res.exec_time_ns
        _cache["trace"] = res.instructions_and_trace

    y = np.zeros((S, HIDDEN), np.float64)
    for c in range(N_CORES):
        y += res.results[c]["y"].astype(np.float64)
    return y.astype(np.float32).reshape(1, S, HIDDEN)


# revision 29
# speedup vs baseline: 1.1186x; 1.1186x over previous
"""Trainium2 Bass kernel for Llama-style GQA attention (nn_LlamaAttention).

Shapes (hardcoded): hidden [1, 2048, 2048] f32, Wq [2048, 2048],
Wk/Wv [512, 2048], Wo [2048, 2048]. 32 q heads, 8 kv heads, head_dim 64,
causal + interleaved RoPE.

Sharding: tensor-parallel over heads across 8 NeuronCores. Core c owns
q heads 4c..4c+3 (one GQA group) and kv head c. Each core computes its
q/k/v projections (output-dim shard), RoPE, causal attention for its 4
heads, and a partial output projection (Wo input-dim shard). The host
sums the 8 partial [s, m] outputs.

v2: all data in bf16 (fp32 matmuls measured 2.6 cyc/row on HW vs 1 for
bf16; also halves HBM traffic). RoPE pair-swap via DVE stream_shuffle
(replaces partition-strided SBUF-SBUF DMAs). Fast-approx reciprocal for
the softmax denominators. Output projection interleaved between the two
attention head-pairs so normalization latency hides under PE work.

On-core dataflow (seq dim on the free axis):
  hT [h, s] -> qT [256, s], kT/vT [64, s]  (bf16 matmuls, N=512 chunks)
  RoPE via stream_shuffle pair-swap + DVE combine (f32 from PSUM)
  scores sT[j, i] = kT^T q, two heads packed in the PE array (K=64 rows)
  P = exp(sT) on ScalarE (no max subtraction; scores are O(1) bounded)
  causal: lower-left block skipping + one triangular 128x128 mask
  O^T accumulation with a ones-column in V to get the softmax denominator
  normalize via reciprocal_approx_fast + gpsimd partition-broadcast
  y[s, m] = O^T^T @ Wo_shard^T partials (bf16), summed on host.
"""

import numpy as np

HIDDEN = 2048
S = 2048
NH = 32
NKV = 8
HD = 64
GROUPS = 4
N_CORES = 8
DQ = 256          # q output dims per core (4 heads x 64)
CH = 512          # seq chunk width
NCH = S // CH     # 4
KT = HIDDEN // 128  # 16 contraction tiles

_cache = {}

# even/odd partition pair swap within each 32-lane quadrant
SWAP_MASK = [i ^ 1 for i in range(32)]
IDENT_MASK = list(range(32))


def _build_program(repeat=1, dumps=False):
    import concourse.bacc as bacc
    import concourse.mybir as mybir
    import concourse.tile as tile

    f32 = mybir.dt.float32
    bf16 = mybir.dt.bfloat16
    EXP = mybir.ActivationFunctionType.Exp

    nc = bacc.Bacc("TRN2", target_bir_lowering=False, debug=False,
                   num_devices=N_CORES)

    hT = nc.declare_dram_parameter("hT", [HIDDEN, S], bf16, isOutput=False)
    wqkvT = nc.declare_dram_parameter("wqkvT", [HIDDEN, DQ + 2 * HD], bf16,
                                      isOutput=False)
    woT = nc.declare_dram_parameter("woT", [DQ, HIDDEN], bf16, isOutput=False)
    tables = nc.declare_dram_parameter("tables", [128, NCH, 4 * CH], f32,
                                       isOutput=False)
    tri2 = nc.declare_dram_parameter("tri2", [128, 256], bf16, isOutput=False)
    onesall = nc.declare_dram_parameter("onesall", [128, KT * 64], bf16,
                                        isOutput=False)
    y = nc.declare_dram_parameter("y", [S, HIDDEN], bf16, isOutput=True)
    if dumps:
        dbg_q = [nc.declare_dram_parameter(f"dbg_q{m}", [128, S], bf16,
                                           isOutput=True) for m in range(2)]
        dbg_k = nc.declare_dram_parameter("dbg_k", [128, S], bf16,
                                          isOutput=True)
        dbg_v = nc.declare_dram_parameter("dbg_v", [128, 192 * KT], bf16,
                                          isOutput=True)
        dbg_ot = [nc.declare_dram_parameter(f"dbg_ot{m}", [128, S], bf16,
                                            isOutput=True) for m in range(2)]
        dbg_pt = nc.declare_dram_parameter("dbg_pt", [128, 4096], bf16,
                                           isOutput=True)
        dbg_otp = nc.declare_dram_parameter("dbg_otp", [128, 1024], f32,
                                            isOutput=True)
        dbg_nrm = nc.declare_dram_parameter("dbg_nrm", [64, 1024], f32,
                                            isOutput=True)

    with tile.TileContext(nc) as tc:
        with (
            tc.tile_pool(name="const", bufs=1) as const,
            tc.tile_pool(name="weights", bufs=1) as wpool,
            tc.tile_pool(name="ht", bufs=1) as htp,
            tc.tile_pool(name="work", bufs=1) as work,
            tc.tile_pool(name="persist", bufs=1) as persist,
            tc.tile_pool(name="vp", bufs=1) as vp,
            tc.tile_pool(name="ptp", bufs=1) as ptp,
            tc.tile_pool(name="ppa", bufs=1, space="PSUM") as ppa,
            tc.tile_pool(name="pps", bufs=1, space="PSUM") as pps,
            tc.tile_pool(name="ppo", bufs=1, space="PSUM") as ppo,
        ):
            # ---- constants / weights ----
            t_tri2 = const.tile([128, 2, 128], bf16)
            nc.sync.dma_start(out=t_tri2, in_=tri2[:, :])

            t_wqkv = wpool.tile([128, KT, DQ + 2 * HD], bf16)
            wqkv_r = wqkvT[:, :].rearrange("(t p) o -> p t o", p=128)

            # persistent activations
            t_q = [persist.tile([128, S], bf16, tag=f"q{m}", name=f"t_q{m}")
                   for m in range(2)]
            t_k = persist.tile([128, S], bf16, tag="k")
            t_ot = [persist.tile([128, S], bf16, tag=f"ot{m}", name=f"t_ot{m}")
                    for m in range(2)]
            # v tiles: [v(0:64) | ones,zeros(64:128) | v(128:192)] per j-block
            t_v = vp.tile([128, KT, 192], bf16, tag="v")
            nc.sync.dma_start(
                out=t_v[:, :, 64:128],
                in_=onesall[:, :].rearrange("p (t o) -> p t o", o=64))

            t_wo = None

            def rope(dst, ps, cos_t, sin_t, rows, ci):
                """dst[:, chunk] = ps*cos + shuffle(ps)*sin (rows slice).

                ps is the PSUM projection result (f32); shuffle swaps
                even/odd partition pairs; sign is folded into sin_t."""
                r0, r1 = rows
                c0 = ci * CH
                swp = work.tile([128, CH], f32, tag="swp", bufs=2)
                nc.vector.stream_shuffle(swp[r0:r1], ps[r0:r1], SWAP_MASK)
                tmp1 = work.tile([128, CH], f32, tag="rc1", bufs=2)
                tmp2 = work.tile([128, CH], f32, tag="rc2", bufs=2)
                nc.vector.tensor_mul(tmp1[r0:r1], ps[r0:r1], cos_t[r0:r1, :])
                nc.vector.tensor_mul(tmp2[r0:r1], swp[r0:r1], sin_t[r0:r1, :])
                nc.vector.tensor_add(dst[r0:r1, c0:c0 + CH], tmp1[r0:r1],
                                     tmp2[r0:r1])

            def emit_A(ci):
                """Projections + RoPE + k/v prep for s-chunk ci."""
                c0 = ci * CH
                t_ht = htp.tile([128, KT, CH], bf16, tag="ht", bufs=2,
                                name=f"ht_{ci}")
                ht_src = hT[:, c0:c0 + CH].rearrange("(t p) s -> p t s", p=128)
                if ci == 0:
                    # interleave weight/activation tiles so the first
                    # matmul starts ~1us in, not after a monolithic DMA
                    for kt in range(KT):
                        nc.sync.dma_start(
                            out=t_wqkv[:, kt, :], in_=wqkv_r[:, kt, :])
                        nc.sync.dma_start(
                            out=t_ht[:, kt, :], in_=ht_src[:, kt, :])
                else:
                    nc.sync.dma_start(out=t_ht, in_=ht_src)
                # rope tables for this chunk: one fused DMA
                t_tab = work.tile([128, 4 * CH], f32, tag="tab", bufs=2)
                nc.sync.dma_start(out=t_tab, in_=tables[:, ci, :])
                t_cosk = t_tab[:, 0:CH]
                t_sink = t_tab[:, CH:2 * CH]
                t_cosq = t_tab[:, 2 * CH:3 * CH]
                t_sinq = t_tab[:, 3 * CH:4 * CH]

                # kv pass first so k/v prep overlaps the q passes
                ps_kv = ppa.tile([128, CH], f32, tag="proj", bufs=2,
                                 name=f"ps_kv{ci}")
                for kt in range(KT):
                    nc.tensor.matmul(ps_kv, t_wqkv[:, kt, 256:384],
                                     t_ht[:, kt, :],
                                     start=(kt == 0), stop=(kt == KT - 1))
                # ---- k: RoPE rows 0:64 straight from PSUM, then dup ----
                rope(t_k, ps_kv, t_cosk, t_sink, (0, 64), ci)
                nc.sync.dma_start(out=t_k[64:128, c0:c0 + CH],
                                  in_=t_k[0:64, c0:c0 + CH])
                # ---- v: bf16 copy, XBAR DMA-transpose [64,128] -> [128,64] --
                v_raw = work.tile([128, CH], bf16, tag="vraw", bufs=2)
                nc.scalar.copy(v_raw[64:128], ps_kv[64:128])
                for b in range(4):
                    jb = 4 * ci + b
                    nc.sync.dma_start_transpose(
                        t_v[:, jb, 0:64],
                        v_raw[64:128, b * 128:(b + 1) * 128])
                    nc.vector.tensor_copy(t_v[:, jb, 128:192],
                                          t_v[:, jb, 0:64])

                # ---- q passes + RoPE (2 partition tiles = 4 heads) ----
                for m in range(2):
                    ps_q = ppa.tile([128, CH], f32, tag="proj", bufs=2,
                                    name=f"ps_q{m}_{ci}")
                    for kt in range(KT):
                        nc.tensor.matmul(
                            ps_q, t_wqkv[:, kt, m * 128:(m + 1) * 128],
                            t_ht[:, kt, :],
                            start=(kt == 0), stop=(kt == KT - 1))
                    rope(t_q[m], ps_q, t_cosq, t_sinq, (0, 128), ci)

            def emit_B_pair(ci, pair, cq=()):
                """Attention for i-chunk ci, one packed head pair.

                cq: queue of output-projection groups (callables) slotted
                one-per-jb after the PV matmuls — their 4 matmuls execute
                on the PE while this jb's exp runs on ScalarE, keeping the
                PE dense (HAM stays at full clock)."""
                cq = list(cq)
                c0 = ci * CH
                if ci == 0 and pair == 0:
                    # Wo halves load during attention (DMA slack window)
                    for p_ in range(2):
                        nc.sync.dma_start(
                            out=t_wo[:, p_, :],
                            in_=woT[:, :].rearrange(
                                "(t p) o -> p t o", p=128)[:, p_, :])
                ot = ppo.tile([128, 2 * CH], f32, tag="ot", bufs=1,
                              name=f"ot{ci}_{pair}")
                njb = 4 * ci + 4
                for jb in range(njb):
                    r = jb - 4 * ci
                    off = 128 * max(r, 0)
                    w = CH - off
                    j0 = jb * 128
                    # both heads' scores in one 2-bank tile -> one ACTIVATE
                    ps_s = pps.tile([128, 2 * CH], f32, tag="s", bufs=2,
                                    name=f"ps_s{ci}_{pair}_{jb}")
                    for h in range(2):
                        nc.tensor.matmul(
                            ps_s[:, h * CH:h * CH + w],
                            t_k[64 * h:64 * (h + 1), j0:j0 + 128],
                            t_q[pair][64 * h:64 * (h + 1),
                                      c0 + off:c0 + CH],
                            start=True, stop=True,
                            tile_position=(64 * h, 0))
                    pt = ptp.tile([128, 2 * CH], bf16, tag="pt", bufs=4)
                    ps3 = ps_s[:, :].rearrange("p (a c) -> p a c", a=2)
                    pt3 = pt[:, :].rearrange("p (a c) -> p a c", a=2)
                    nc.scalar.activation(pt3[:, :, 0:w], ps3[:, :, 0:w], EXP)
                    if r >= 0:
                        nc.vector.tensor_mul(pt3[:, :, 0:128],
                                             pt3[:, :, 0:128], t_tri2)
                    if dumps and ci == 0 and pair == 0:
                        for h in range(2):
                            nc.sync.dma_start(
                                out=dbg_pt[:, h * 2048 + jb * 512:
                                           h * 2048 + jb * 512 + w],
                                in_=pt[:, h * CH:h * CH + w])
                    nc.tensor.matmul(
                        ot[0:65, off:CH],
                        t_v[:, jb, 0:65], pt[:, 0:w],
                        start=(jb == 0), stop=(jb == njb - 1))
                    nc.tensor.matmul(
                        ot[0:128, CH + off:2 * CH],
                        t_v[:, jb, 64:192], pt[:, CH:CH + w],
                        start=(jb == 0), stop=(jb == njb - 1))
                    if cq and jb % max(1, njb // 4) == 0:
                        cq.pop(0)()
                while cq:
                    cq.pop(0)()
                if dumps and ci == 0 and pair == 0:
                    otp_b = work.tile([128, 2 * CH], f32, tag="otdbg", bufs=1)
                    nc.scalar.copy(otp_b, ot)
                    nc.sync.dma_start(out=dbg_otp[:, :], in_=otp_b)
                # normalize: stage each head's denominator row to a
                # partition-0-based SBUF tile (partition_broadcast and the
                # custom recip op need base partition 0), then broadcast
                for h in range(2):
                    stg = work.tile([32, CH], f32, tag="stage", bufs=4)
                    if h == 0:
                        # den row lives at PSUM partition 64: quadrant move
                        nc.vector.stream_shuffle(stg, ot[64:96, 0:CH],
                                                 IDENT_MASK)
                    else:
                        nc.vector.tensor_copy(stg[0:1], ot[0:1, CH:2 * CH])
                    recip = work.tile([32, CH], f32, tag="recip", bufs=4)
                    nc.vector.reciprocal_approx_fast(recip, stg)
                    if dumps and ci == 0 and pair == 0:
                        nc.sync.dma_start(out=dbg_nrm[32 * h:32 * h + 32,
                                                      0:CH], in_=stg)
                        nc.sync.dma_start(out=dbg_nrm[32 * h:32 * h + 32,
                                                      CH:2 * CH], in_=recip)
                    bcast = work.tile([128, CH], f32, tag="bcast", bufs=2)
                    nc.gpsimd.partition_broadcast(bcast, recip[0:1, :])
                    r0_, r1_ = (0, 64) if h == 0 else (64, 128)
                    o_sl = (ot[0:64, 0:CH] if h == 0
                            else ot[64:128, CH:2 * CH])
                    nc.vector.tensor_mul(
                        t_ot[pair][r0_:r1_, c0:c0 + CH],
                        o_sl, bcast[r0_:r1_, :])

            def emit_C_group(st, mc0):
                """One output-projection group: y[st-block, mc0..mc0+2).
                mc handled two-at-a-time so each OT weight load serves two
                matmuls (halves LDWEIGHTS traffic)."""
                ps_y = pps.tile([128, 2 * CH], f32, tag="s", bufs=2,
                                name=f"ps_y{st}_{mc0}")
                for k in range(2):
                    for j in range(2):
                        mc = mc0 + j
                        nc.tensor.matmul(
                            ps_y[:, j * CH:(j + 1) * CH],
                            t_ot[k][:, st * 128:(st + 1) * 128],
                            t_wo[:, k, mc * CH:(mc + 1) * CH],
                            start=(k == 0), stop=(k == 1))
                t_y = work.tile([128, 2 * CH], bf16, tag="ybounce",
                                bufs=3)
                nc.vector.tensor_copy(t_y, ps_y)
                nc.sync.dma_start(
                    out=y[st * 128:(st + 1) * 128,
                          mc0 * CH:(mc0 + 2) * CH],
                    in_=t_y)

            def c_groups(ci):
                import functools
                return [functools.partial(emit_C_group, st, mc0)
                        for st in range(4 * ci, 4 * ci + 4)
                        for mc0 in range(0, HIDDEN // CH, 2)]

            # Software-pipelined emission: A(ci+1) hides RoPE chains; the
            # output projection C(ci-1) is split into 8 groups slotted
            # into B's jb loops so the PE stays dense while each jb's exp
            # runs on ScalarE (keeps HAM at full clock).
            t_wo = wpool.tile([128, 2, HIDDEN], bf16, tag="wo")
            for rep in range(repeat):
                emit_A(0)
                for ci in range(NCH):
                    if ci + 1 < NCH:
                        emit_A(ci + 1)
                    cg = c_groups(ci - 1) if ci >= 1 else []
                    emit_B_pair(ci, 0, cg[:4])
                    emit_B_pair(ci, 1, cg[4:])
                for g in c_groups(NCH - 1):
                    g()

            if dumps:
                for m in range(2):
                    nc.sync.dma_start(out=dbg_q[m][:, :], in_=t_q[m][:, :])
                    nc.sync.dma_start(out=dbg_ot[m][:, :], in_=t_ot[m][:, :])
                nc.sync.dma_start(out=dbg_k[:, :], in_=t_k[:, :])
                nc.sync.dma_start(
                    out=dbg_v[:, :].rearrange("p (t o) -> p t o", o=192),
                    in_=t_v[:, :, :])

    nc.compile()
    return nc


def _host_inputs(hidden_states, Wq, Wk, Wv, Wo):
    import ml_dtypes
    bf16 = ml_dtypes.bfloat16

    hid = np.ascontiguousarray(hidden_states.reshape(S, HIDDEN),
                               dtype=np.float32)
    hT = np.ascontiguousarray(hid.T).astype(bf16)

    scale = HD ** -0.5
    inv = 1.0 / (10000.0 ** (np.arange(0, HD, 2, dtype=np.float64) / HD))
    t = np.arange(S, dtype=np.float64)
    freqs = np.outer(t, inv)                       # [S, 32]
    cos_sd = np.repeat(np.cos(freqs), 2, axis=1)   # [S, 64]
    sin_sd = np.repeat(np.sin(freqs), 2, axis=1)
    sign = np.tile(np.array([-1.0, 1.0]), HD // 2)
    cosT = cos_sd.T                                # [64, S]
    sinT = (sin_sd * sign).T
    cosk = np.concatenate([cosT, cosT], 0).astype(np.float32)
    sink = np.concatenate([sinT, sinT], 0).astype(np.float32)
    cosq = (cosk * scale).astype(np.float32)
    sinq = (sink * scale).astype(np.float32)

    tabs = np.zeros((128, NCH, 4 * CH), np.float32)
    for ci in range(NCH):
        sl = slice(ci * CH, (ci + 1) * CH)
        tabs[:, ci, 0:CH] = cosk[:, sl]
        tabs[:, ci, CH:2 * CH] = sink[:, sl]
        tabs[:, ci, 2 * CH:3 * CH] = cosq[:, sl]
        tabs[:, ci, 3 * CH:4 * CH] = sinq[:, sl]

    tri = np.triu(np.ones((128, 128), np.float32))
    tri2 = np.concatenate([tri, tri], axis=1).astype(bf16)   # [128, 256]
    onesall = np.zeros((128, KT, 64), np.float32)
    onesall[:, :, 0] = 1.0
    onesall = onesall.reshape(128, KT * 64).astype(bf16)

    in_maps = []
    for c in range(N_CORES):
        wq_c = Wq[DQ * c:DQ * (c + 1), :]          # [256, H]
        wk_c = Wk[HD * c:HD * (c + 1), :]          # [64, H]
        wv_c = Wv[HD * c:HD * (c + 1), :]
        wqkvT = np.ascontiguousarray(
            np.concatenate([wq_c, wk_c, wv_c], axis=0).T).astype(bf16)
        woT = np.ascontiguousarray(
            Wo[:, DQ * c:DQ * (c + 1)].T).astype(bf16)
        in_maps.append({
            "hT": hT, "wqkvT": wqkvT, "woT": woT,
            "tables": tabs,
            "tri2": tri2, "onesall": onesall,
        })
    return in_maps


def kernel(hidden_states, Wq, Wk, Wv, Wo):
    import os
    from concourse.bass_utils import run_bass_kernel_spmd

    if "nc" not in _cache:
        _cache["nc"] = _build_program()
    nc = _cache["nc"]

    trace = bool(os.environ.get("BASS_HW_TRACE"))
    in_maps = _host_inputs(hidden_states, Wq, Wk, Wv, Wo)
    res = run_bass_kernel_spmd(nc, in_maps, list(range(N_CORES)),
                               trace=trace)
    if trace:
        _cache["exec_time_ns"] = res.exec_time_ns
        _cache["trace"] = res.instructions_and_trace

    y = np.zeros((S, HIDDEN), np.float64)
    for c in range(N_CORES):
        y += res.results[c]["y"].astype(np.float64)
    return y.astype(np.float32).reshape(1, S, HIDDEN)
